# revision 1
# baseline (speedup 1.0000x reference)
"""BitNet attention layer (quantized QKV + attention + quantized dense + LN)
as a Bass/Tile SPMD kernel for 8 Trainium2 NeuronCores.

Sharding: core c = 2*b + g handles batch b (of 4) and head-group g (of 2,
8 heads each).  QKV projection + attention are fully local per core
(tensor-parallel over heads, data-parallel over batch); the dense output
projection is tensor-parallel over its input dim, pair-reduced with a
ReduceScatter so each core finishes residual+layernorm on its half of the
batch's tokens.  Cross-core scalars (weight abs-means, activation abs-maxes)
use tiny AllReduce collectives.

All matmuls run in bf16.  The BitNet quantization makes the two projection
matmuls *exact*: activations are round()ed to integers in [-127,127] and
weights to {-1,0,1}, both exactly representable in bf16, and fp32 PSUM
accumulation of <=2048 such products is exact.  Rounding uses the
+2^23-then-subtract trick on the fp32 DVE datapath, which reproduces
numpy/jax round-half-to-even semantics.
"""

import math
import os
import sys

import numpy as np

sys.path.insert(0, "/opt/trn_rl_repo")

import concourse.bacc as bacc
import concourse.bass as bass
import concourse.bass_isa as bass_isa
import concourse.mybir as mybir
import concourse.tile as tile

F32 = mybir.dt.float32
BF16 = mybir.dt.bfloat16
AF = mybir.ActivationFunctionType
OP = mybir.AluOpType

P = 128
H = 2048
S = 2048
B = 4
NH = 16
HD = 128
NCORES = 8
TOK = S                # tokens per batch
HB = H // P            # 16 h blocks
NHC = NH // 2          # 8 heads per core
HALF = TOK // 2        # 1024 tokens per core after reduce-scatter
MAGIC = float(2 ** 23)
INV_SQD = 1.0 / math.sqrt(HD)
LN_EPS = 1e-5
PAIRS = [[0, 1], [2, 3], [4, 5], [6, 7]]
ALL8 = [list(range(NCORES))]
CH = 4                 # 512-wide free chunks over TOK
CW = TOK // CH


def _chunks(count, width):
    return [(i, slice(i * width, (i + 1) * width)) for i in range(count)]


def build_program(use_mask: bool, qk_bias_zero: bool, v_bias_zero: bool,
                  d_bias_zero: bool, ln_trivial: bool):
    nc = bacc.Bacc("TRN2", target_bir_lowering=False, debug=False,
                   enable_asserts=False, num_devices=NCORES)

    # ---- I/O --------------------------------------------------------------
    xt = nc.dram_tensor("xt", [H, TOK], F32, kind="ExternalInput")
    xr = nc.dram_tensor("xr", [HALF, H], F32, kind="ExternalInput")
    wqkt = nc.dram_tensor("wqkt", [H, 2048], F32, kind="ExternalInput")
    wvt = nc.dram_tensor("wvt", [H, 1024], F32, kind="ExternalInput")
    bqk = nc.dram_tensor("bqk", [P, 16], F32, kind="ExternalInput")
    bv = nc.dram_tensor("bv", [1, 1024], F32, kind="ExternalInput")
    wdt = nc.dram_tensor("wdt", [HALF, H], F32, kind="ExternalInput")
    bdh = nc.dram_tensor("bdh", [1, H], F32, kind="ExternalInput")
    maskt = nc.dram_tensor("maskt", [P, HB], F32, kind="ExternalInput")
    lnw = nc.dram_tensor("lnw", [1, H], F32, kind="ExternalInput")
    lnb = nc.dram_tensor("lnb", [1, H], F32, kind="ExternalInput")
    out = nc.dram_tensor("out", [HALF, H], F32, kind="ExternalOutput")

    # ---- DRAM scratch ----------------------------------------------------
    wq_q = nc.dram_tensor("wq_q", [H, 2048], BF16)
    wv_q = nc.dram_tensor("wv_q", [H, 1024], BF16)
    wd_q = nc.dram_tensor("wd_q", [HALF, H], BF16)
    qkt_d = nc.dram_tensor("qkt_d", [16, P, TOK], BF16)
    ctxn_d = nc.dram_tensor("ctxn_d", [NHC, P, TOK], F32)
    rs_in = nc.dram_tensor("rs_in", [TOK, H], F32)
    rs_out = nc.dram_tensor("rs_out", [HALF, H], F32)
    c_add_i = nc.dram_tensor("c_add_i", [1, 16], F32)
    c_add_o = nc.dram_tensor("c_add_o", [1, 16], F32)
    c_mx_i = nc.dram_tensor("c_mx_i", [1, 16], F32)
    c_mx_o = nc.dram_tensor("c_mx_o", [1, 16], F32)
    c_mc_i = nc.dram_tensor("c_mc_i", [1, 16], F32)
    c_mc_o = nc.dram_tensor("c_mc_o", [1, 16], F32)

    with tile.TileContext(nc) as tc:
        _emit(tc, locals(), use_mask, qk_bias_zero, v_bias_zero,
              d_bias_zero, ln_trivial)

    nc.compile()
    return nc


def _emit(tc, T, use_mask, qk_bias_zero, v_bias_zero, d_bias_zero, ln_trivial):
    nc = tc.nc
    xt, xr, wqkt, wvt, bqk, bv, wdt, bdh = (T["xt"], T["xr"], T["wqkt"],
                                            T["wvt"], T["bqk"], T["bv"],
                                            T["wdt"], T["bdh"])
    maskt, lnw, lnb, out = T["maskt"], T["lnw"], T["lnb"], T["out"]
    wq_q, wv_q, wd_q, qkt_d, ctxn_d = (T["wq_q"], T["wv_q"], T["wd_q"],
                                       T["qkt_d"], T["ctxn_d"])
    rs_in, rs_out = T["rs_in"], T["rs_out"]
    c_add_i, c_add_o = T["c_add_i"], T["c_add_o"]
    c_mx_i, c_mx_o = T["c_mx_i"], T["c_mx_o"]
    c_mc_i, c_mc_o = T["c_mc_i"], T["c_mc_o"]

    from contextlib import ExitStack

    est = ExitStack()
    with est:
        # (mid is closed explicitly after stage 2)
        # Long-lived pools.
        smalls = est.enter_context(tc.tile_pool(name="smalls", bufs=1))
        stream = est.enter_context(tc.tile_pool(name="stream", bufs=3))
        red = est.enter_context(tc.tile_pool(name="red", bufs=4))
        ps = est.enter_context(tc.tile_pool(name="ps", bufs=2, space="PSUM"))
        mid = ExitStack()
        vt_pool = mid.enter_context(tc.tile_pool(name="vt", bufs=16))

        def sc_tile(name, shape=(1, 1)):
            return smalls.tile(list(shape), F32, tag=name, name=name)

        ones_col = smalls.tile([P, 1], BF16, tag="ones_col")
        bqk_sb = smalls.tile([P, 16], F32, tag="bqk_sb")
        if os.environ.get("NOONES") != "1":
            nc.vector.memset(ones_col[:], 1.0)
            nc.sync.dma_start(bqk_sb[:], bqk[:, :])
        mask_sb = None
        if use_mask:
            mask_sb = smalls.tile([P, HB], F32, tag="mask_sb")
            nc.sync.dma_start(mask_sb[:], maskt[:, :])

        if os.environ.get("BYPASS0") == "1":
            gq = sc_tile("gq"); nc.vector.memset(gq[:], 0.016)
            gd = sc_tile("gd"); nc.vector.memset(gd[:], 0.016)
            igq_b = sc_tile("igq_b", (P, 1)); nc.vector.memset(igq_b[:], 62.5)
            igd_b = sc_tile("igd_b", (P, 1)); nc.vector.memset(igd_b[:], 62.5)
            sx_b = sc_tile("sx_b", (P, 1)); nc.vector.memset(sx_b[:], 25.0)
            alpha_b = sc_tile("alpha_b", (P, 1)); nc.vector.memset(alpha_b[:], 0.00064)
        else:
            # ================= Stage 0a: |W| partial sums ======================
            accA = sc_tile("accA", (P, 1))
            accB = sc_tile("accB", (P, 1))

            absdump = smalls.tile([P, 2048], F32, tag="absdump")

            def abs_sum_into(dram_ap, nrows, width, acc, first):
                for t in range(nrows // P):
                    wf = stream.tile([P, width], F32, tag="st32")
                    nc.sync.dma_start(wf[:], dram_ap[t * P:(t + 1) * P, :])
                    r = red.tile([P, 1], F32, tag="wred")
                    nc.scalar.activation(absdump[:, :width], wf[:], AF.Abs,
                                         accum_out=r[:])
                    if first and t == 0:
                        nc.vector.tensor_copy(acc[:], r[:])
                    else:
                        nc.vector.tensor_tensor(acc[:], acc[:], r[:], OP.add)

            abs_sum_into(wqkt, H, 2048, accA, True)
            abs_sum_into(wvt, H, 1024, accA, False)
            abs_sum_into(wdt, HALF, H, accB, True)

            accAr = sc_tile("accAr", (P, 1))
            accBr = sc_tile("accBr", (P, 1))
            nc.gpsimd.partition_all_reduce(accAr[:], accA[:], channels=P,
                                           reduce_op=bass_isa.ReduceOp.add)
            nc.gpsimd.partition_all_reduce(accBr[:], accB[:], channels=P,
                                           reduce_op=bass_isa.ReduceOp.add)
            zpad = sc_tile("zpad", (1, 16))
            nc.vector.memset(zpad[:], 0.0)
            nc.vector.tensor_copy(zpad[0:1, 0:1], accAr[0:1, 0:1])
            nc.vector.tensor_copy(zpad[0:1, 1:2], accBr[0:1, 0:1])
            nc.gpsimd.dma_start(c_add_i[:, :], zpad[:])
            nc.gpsimd.collective_compute(
                "AllReduce", OP.add, replica_groups=PAIRS,
                ins=[c_add_i[:, :].opt()], outs=[c_add_o[:, :].opt()])
            wsums = sc_tile("wsums", (1, 16))
            nc.sync.dma_start(wsums[:], c_add_o[:, :])

            # gamma_qkv = sum/|W_qkv| count + 1e-5 ; gamma_d likewise
            gq = sc_tile("gq")
            nc.vector.tensor_scalar(gq[:], wsums[0:1, 0:1],
                                    1.0 / (3 * H * H), 1e-5, OP.mult, OP.add)
            igq = sc_tile("igq")
            nc.vector.reciprocal(igq[:], gq[:])
            gd = sc_tile("gd")
            nc.vector.tensor_scalar(gd[:], wsums[0:1, 1:2],
                                    1.0 / (H * H), 1e-5, OP.mult, OP.add)
            igd = sc_tile("igd")
            nc.vector.reciprocal(igd[:], gd[:])

            igq_b = sc_tile("igq_b", (P, 1))
            nc.gpsimd.partition_broadcast(igq_b[:], igq[:])
            igd_b = sc_tile("igd_b", (P, 1))
            nc.gpsimd.partition_broadcast(igd_b[:], igd[:])

            # ================= Stage 0b: max|x| ================================
            xmax = sc_tile("xmax", (P, 1))
            for t in range(HB):
                xf = stream.tile([P, TOK], F32, tag="st32")
                nc.sync.dma_start(xf[:], xt[t * P:(t + 1) * P, :])
                r = red.tile([P, 1], F32, tag="xred")
                nc.vector.tensor_reduce(r[:], xf[:], axis=mybir.AxisListType.X,
                                        op=OP.max, apply_absolute_value=True)
                if t == 0:
                    nc.vector.tensor_copy(xmax[:], r[:])
                else:
                    nc.vector.tensor_tensor(xmax[:], xmax[:], r[:], OP.max)
            xmaxr = sc_tile("xmaxr", (P, 1))
            nc.gpsimd.partition_all_reduce(xmaxr[:], xmax[:], channels=P,
                                           reduce_op=bass_isa.ReduceOp.max)
            zpad2 = sc_tile("zpad2", (1, 16))
            nc.vector.memset(zpad2[:], 0.0)
            nc.vector.tensor_copy(zpad2[0:1, 0:1], xmaxr[0:1, 0:1])
            nc.gpsimd.dma_start(c_mx_i[:, :], zpad2[:])
            nc.gpsimd.collective_compute(
                "AllReduce", OP.max, replica_groups=ALL8,
                ins=[c_mx_i[:, :].opt()], outs=[c_mx_o[:, :].opt()])
            xm = sc_tile("xm", (1, 16))
            nc.sync.dma_start(xm[:], c_mx_o[:, :])

            xm1 = sc_tile("xm1")
            nc.vector.tensor_scalar(xm1[:], xm[0:1, 0:1], 1e-8, None, OP.add)
            rxm = sc_tile("rxm")
            nc.vector.reciprocal(rxm[:], xm1[:])
            sx = sc_tile("sx")
            nc.vector.tensor_scalar(sx[:], rxm[:], 127.0, None, OP.mult)
            # alpha_qkv = gamma_q / s_x = gamma_q * (max+1e-8) / 127
            al_t = sc_tile("al_t")
            nc.vector.tensor_tensor(al_t[:], gq[:], xm1[:], OP.mult)
            alpha = sc_tile("alpha")
            nc.vector.tensor_scalar(alpha[:], al_t[:], 1.0 / 127.0, None, OP.mult)
            sx_b = sc_tile("sx_b", (P, 1))
            nc.gpsimd.partition_broadcast(sx_b[:], sx[:])
            alpha_b = sc_tile("alpha_b", (P, 1))
            nc.gpsimd.partition_broadcast(alpha_b[:], alpha[:])


        bvb = None
        if not v_bias_zero:
            bv_sb = smalls.tile([1, 1024], F32, tag="bv_sb")
            nc.sync.dma_start(bv_sb[:], bv[:, :])
            bvb = smalls.tile([P, 1024], F32, tag="bvb")
            nc.gpsimd.partition_broadcast(bvb[:], bv_sb[:])

        # ================= Stage 0c: quantize weights to DRAM (bf16) =======
        def quantize_w(dram_in, dram_out, nrows, width, inv_gamma_b):
            for t in range(nrows // P):
                wf = stream.tile([P, width], F32, tag="st32")
                nc.sync.dma_start(wf[:], dram_in[t * P:(t + 1) * P, :])
                t1 = stream.tile([P, width], F32, tag="st32")
                nc.vector.tensor_scalar(t1[:], wf[:], inv_gamma_b[:], MAGIC,
                                        OP.mult, OP.add)
                eng = nc.vector if t % 2 == 0 else nc.gpsimd
                t2 = stream.tile([P, width], F32, tag="st32")
                eng.tensor_scalar(t2[:], t1[:], MAGIC, 1.0,
                                  OP.subtract, OP.min)
                t3 = stream.tile([P, width], BF16, tag="st16")
                eng.tensor_scalar(t3[:], t2[:], -1.0, None, OP.max)
                nc.sync.dma_start(dram_out[t * P:(t + 1) * P, :], t3[:])

        if os.environ.get("SKIP0C") != "1":
            quantize_w(wqkt, wq_q, H, 2048, igq_b)
            quantize_w(wvt, wv_q, H, 1024, igq_b)
            quantize_w(wdt, wd_q, HALF, H, igd_b)

        # ================= Stage 1: quantize x, QKV projection =============
        with tc.tile_pool(name="xq", bufs=HB) as xq_pool, \
             tc.tile_pool(name="wv_sb", bufs=HB) as wv_pool, \
             tc.tile_pool(name="s1ev", bufs=2) as ev_pool, \
             tc.tile_pool(name="s1l", bufs=2 * HB) as l_pool:

            xq = []
            for kb in range(HB):
                q = xq_pool.tile([P, TOK], BF16, tag="xq")
                if os.environ.get("XQMEMSET") == "1":
                    nc.vector.memset(q[:], 1.0)
                else:
                    xf = stream.tile([P, TOK], F32, tag="st32")
                    nc.sync.dma_start(xf[:], xt[kb * P:(kb + 1) * P, :])
                    t1 = stream.tile([P, TOK], F32, tag="st32")
                    nc.vector.tensor_scalar(t1[:], xf[:], sx_b[:], MAGIC,
                                            OP.mult, OP.add)
                    enx = nc.vector if kb % 2 == 0 else nc.gpsimd
                    enx.tensor_scalar(q[:], t1[:], MAGIC, None,
                                      OP.subtract)
                xq.append(q)

            # Q^T and K^T, one 128-row output block at a time -> DRAM bf16.
            for ob in range(int(os.environ.get("NOBS", "16"))):
                psum = ps.tile([P, TOK], F32, tag="ps")
                lts = []
                for kb in range(HB):
                    lt = l_pool.tile([P, P], BF16, tag="lt")
                    if os.environ.get("LTCONT") == "1":
                        nc.sync.dma_start(lt[:], wq_q[kb * P:(kb + 1) * P, 0:P])
                    else:
                        nc.sync.dma_start(
                            lt[:], wq_q[kb * P:(kb + 1) * P, ob * P:(ob + 1) * P])
                    lts.append(lt)
                for kb in range(HB):
                    for c, sl in _chunks(CH, CW):
                        nc.tensor.matmul(psum[:, sl], lhsT=lts[kb][:],
                                         rhs=xq[kb][:, sl],
                                         start=(kb == 0), stop=(kb == HB - 1))
                ev = ev_pool.tile([P, TOK], BF16, tag="ev")
                if os.environ.get("EVICT", "act") == "dve":
                    nc.vector.tensor_scalar(ev[:], psum[:], alpha_b[:],
                                            None, OP.mult)
                elif qk_bias_zero:
                    nc.scalar.activation(ev[:], psum[:], AF.Identity,
                                         bias=0.0, scale=alpha_b[:])
                else:
                    nc.scalar.activation(ev[:], psum[:], AF.Identity,
                                         bias=bqk_sb[:, ob:ob + 1],
                                         scale=alpha_b[:])
                if os.environ.get("NODMA") != "1":
                    nc.sync.dma_start(qkt_d[ob, :, :], ev[:])

            # V: [tok, 1024] per token block, kept in SBUF bf16.
            wv_list = []
            if os.environ.get("NOWV") != "1":
                for kb in range(HB):
                    wvq = wv_pool.tile([P, 1024], BF16, tag="wv_sb")
                    nc.sync.dma_start(wvq[:], wv_q[kb * P:(kb + 1) * P, :])
                    wv_list.append(wvq)
            vt = []
            for tb in range(int(os.environ.get("NTBS", str(HB)))):
                psum_full = ps.tile([P, TOK], F32, tag="ps")
                psum = psum_full[:, 0:1024]
                for kb in range(HB):
                    for c in range(2):
                        sl = slice(c * 512, (c + 1) * 512)
                        nc.tensor.matmul(
                            psum[:, sl],
                            lhsT=xq[kb][:, tb * P:(tb + 1) * P],
                            rhs=wv_list[kb][:, sl],
                            start=(kb == 0), stop=(kb == HB - 1))
                v = vt_pool.tile([P, 1024], BF16, tag="vt")
                if v_bias_zero:
                    nc.vector.tensor_scalar(v[:], psum[:], alpha_b[:],
                                            None, OP.mult)
                else:
                    nc.vector.scalar_tensor_tensor(v[:], psum[:], alpha_b[:],
                                                   bvb[:], OP.mult, OP.add)
                vt.append(v)

        if os.environ.get("KSTOP") == "1":
            mid.close()
            return
        # ================= Stage 2: attention ==============================
        mxacc = sc_tile("mxacc", (P, 1))
        with tc.tile_pool(name="qkt", bufs=4) as qk_pool, \
             tc.tile_pool(name="et", bufs=HB) as et_pool, \
             tc.tile_pool(name="rb", bufs=1) as rb_pool, \
             tc.tile_pool(name="cn", bufs=2) as cn_pool, \
             tc.tile_pool(name="rd", bufs=1) as rd_pool:
            for h in range(NHC):
                qt = qk_pool.tile([P, TOK], BF16, tag="qt")
                nc.sync.dma_start(qt[:], qkt_d[h, :, :])
                kt = qk_pool.tile([P, TOK], BF16, tag="kt")
                nc.sync.dma_start(kt[:], qkt_d[NHC + h, :, :])

                et = []
                for kb in range(HB):
                    psum = ps.tile([P, TOK], F32, tag="ps")
                    for c, sl in _chunks(CH, CW):
                        nc.tensor.matmul(psum[:, sl],
                                         lhsT=kt[:, kb * P:(kb + 1) * P],
                                         rhs=qt[:, sl],
                                         start=True, stop=True)
                    e = et_pool.tile([P, TOK], BF16, tag="et")
                    nc.scalar.activation(
                        e[:], psum[:], AF.Exp,
                        bias=(mask_sb[:, kb:kb + 1] if use_mask else 0.0),
                        scale=INV_SQD)
                    et.append(e)

                psc = ps.tile([P, TOK], F32, tag="ps")
                psd = ps.tile([P, TOK], F32, tag="ps")
                for kb in range(HB):
                    vv = vt[kb][:, h * P:(h + 1) * P]
                    for c, sl in _chunks(CH, CW):
                        nc.tensor.matmul(psc[:, sl], lhsT=vv, rhs=et[kb][:, sl],
                                         start=(kb == 0), stop=(kb == HB - 1))
                    for c, sl in _chunks(CH, CW):
                        nc.tensor.matmul(psd[0:1, sl], lhsT=ones_col[:],
                                         rhs=et[kb][:, sl],
                                         start=(kb == 0), stop=(kb == HB - 1))

                rd = rd_pool.tile([1, TOK], F32, tag="rd")
                nc.vector.reciprocal(rd[:], psd[0:1, :])
                rb = rb_pool.tile([P, TOK], F32, tag="rb")
                nc.gpsimd.partition_broadcast(rb[:], rd[:])
                cn = cn_pool.tile([P, TOK], F32, tag="cn")
                nc.vector.tensor_tensor(cn[:], psc[:], rb[:], OP.mult)

                r = red.tile([P, 1], F32, tag="cmax")
                nc.vector.tensor_reduce(r[:], cn[:], axis=mybir.AxisListType.X,
                                        op=OP.max, apply_absolute_value=True)
                if h == 0:
                    nc.vector.tensor_copy(mxacc[:], r[:])
                else:
                    nc.vector.tensor_tensor(mxacc[:], mxacc[:], r[:], OP.max)
                nc.sync.dma_start(ctxn_d[h, :, :], cn[:])

        if os.environ.get("KSTOP") == "2":
            mid.close()
            return
        mid.close()
        mxr = sc_tile("mxr", (P, 1))
        nc.gpsimd.partition_all_reduce(mxr[:], mxacc[:], channels=P,
                                       reduce_op=bass_isa.ReduceOp.max)
        zpad3 = sc_tile("zpad3", (1, 16))
        nc.vector.memset(zpad3[:], 0.0)
        nc.vector.tensor_copy(zpad3[0:1, 0:1], mxr[0:1, 0:1])
        nc.gpsimd.dma_start(c_mc_i[:, :], zpad3[:])
        nc.gpsimd.collective_compute(
            "AllReduce", OP.max, replica_groups=ALL8,
            ins=[c_mc_i[:, :].opt()], outs=[c_mc_o[:, :].opt()])
        cm = sc_tile("cm", (1, 16))
        nc.sync.dma_start(cm[:], c_mc_o[:, :])

        cm1 = sc_tile("cm1")
        nc.vector.tensor_scalar(cm1[:], cm[0:1, 0:1], 1e-8, None, OP.add)
        rcm = sc_tile("rcm")
        nc.vector.reciprocal(rcm[:], cm1[:])
        sctx = sc_tile("sctx")
        nc.vector.tensor_scalar(sctx[:], rcm[:], 127.0, None, OP.mult)
        ad_t = sc_tile("ad_t")
        nc.vector.tensor_tensor(ad_t[:], gd[:], cm1[:], OP.mult)
        alphad = sc_tile("alphad")
        nc.vector.tensor_scalar(alphad[:], ad_t[:], 1.0 / 127.0, None, OP.mult)
        sctx_b = sc_tile("sctx_b", (P, 1))
        nc.gpsimd.partition_broadcast(sctx_b[:], sctx[:])
        alphad_b = sc_tile("alphad_b", (P, 1))
        nc.gpsimd.partition_broadcast(alphad_b[:], alphad[:])

        # quantize ctx -> bf16 ints in SBUF
        cq_pool = est.enter_context(tc.tile_pool(name="cq", bufs=8))
        ctxq = []
        for h in range(NHC):
            cf = stream.tile([P, TOK], F32, tag="st32")
            nc.sync.dma_start(cf[:], ctxn_d[h, :, :])
            t1 = stream.tile([P, TOK], F32, tag="st32")
            nc.vector.tensor_scalar(t1[:], cf[:], sctx_b[:], MAGIC,
                                    OP.mult, OP.add)
            q = cq_pool.tile([P, TOK], BF16, tag="cq")
            enc = nc.vector if h % 2 == 0 else nc.gpsimd
            enc.tensor_scalar(q[:], t1[:], MAGIC, None, OP.subtract)
            ctxq.append(q)

        if os.environ.get("KSTOP") == "25":
            return
        # ================= Stage 3: dense, reduce-scatter, LN ==============
        bdb = None
        if not d_bias_zero:
            bd_sb = smalls.tile([1, H], F32, tag="bd_sb")
            nc.sync.dma_start(bd_sb[:], bdh[:, :])
            bdb = smalls.tile([P, H], F32, tag="bdb")
            nc.gpsimd.partition_broadcast(bdb[:], bd_sb[:])

        with tc.tile_pool(name="wd_sb", bufs=NHC) as wd_pool, \
             tc.tile_pool(name="s3ev", bufs=3) as ev3_pool:
            wd_sb = []
            for kb in range(NHC):
                w = wd_pool.tile([P, H], BF16, tag="wd_sb")
                nc.sync.dma_start(w[:], wd_q[kb * P:(kb + 1) * P, :])
                wd_sb.append(w)
            for tb in range(HB):
                psum = ps.tile([P, TOK], F32, tag="ps")
                for kb in range(NHC):
                    for c, sl in _chunks(CH, CW):
                        nc.tensor.matmul(
                            psum[:, sl],
                            lhsT=ctxq[kb][:, tb * P:(tb + 1) * P],
                            rhs=wd_sb[kb][:, sl],
                            start=(kb == 0), stop=(kb == NHC - 1))
                ev = ev3_pool.tile([P, TOK], F32, tag="ev3")
                if d_bias_zero:
                    nc.vector.tensor_scalar(ev[:], psum[:], alphad_b[:],
                                            None, OP.mult)
                else:
                    nc.vector.scalar_tensor_tensor(ev[:], psum[:], alphad_b[:],
                                                   bdb[:], OP.mult, OP.add)
                nc.sync.dma_start(rs_in[tb * P:(tb + 1) * P, :], ev[:])

        nc.gpsimd.collective_compute(
            "ReduceScatter", OP.add, replica_groups=PAIRS,
            ins=[rs_in[:, :].opt()], outs=[rs_out[:, :].opt()])

        lnwb = lnbb = None
        if not ln_trivial:
            lnw_sb = smalls.tile([1, H], F32, tag="lnw_sb")
            nc.sync.dma_start(lnw_sb[:], lnw[:, :])
            lnwb = smalls.tile([P, H], F32, tag="lnwb")
            nc.gpsimd.partition_broadcast(lnwb[:], lnw_sb[:])
            lnb_sb = smalls.tile([1, H], F32, tag="lnb_sb")
            nc.sync.dma_start(lnb_sb[:], lnb[:, :])
            lnbb = smalls.tile([P, H], F32, tag="lnbb")
            nc.gpsimd.partition_broadcast(lnbb[:], lnb_sb[:])

        with tc.tile_pool(name="ln", bufs=2) as ln_pool, \
             tc.tile_pool(name="lns", bufs=4) as lns_pool:
            for tb in range(HALF // P):
                r_t = ln_pool.tile([P, H], F32, tag="lnr")
                nc.sync.dma_start(r_t[:], rs_out[tb * P:(tb + 1) * P, :])
                x_t = ln_pool.tile([P, H], F32, tag="lnx")
                nc.sync.dma_start(x_t[:], xr[tb * P:(tb + 1) * P, :])

                y = ln_pool.tile([P, H], F32, tag="lny")
                ysum = lns_pool.tile([P, 1], F32, tag="ysum")
                nc.vector.scalar_tensor_tensor(y[:], r_t[:], 0.0, x_t[:],
                                               OP.add, OP.add,
                                               accum_out=ysum[:])
                mu = lns_pool.tile([P, 1], F32, tag="mu")
                nc.vector.tensor_scalar(mu[:], ysum[:], 1.0 / H, None, OP.mult)
                nmu = lns_pool.tile([P, 1], F32, tag="nmu")
                nc.vector.tensor_scalar(nmu[:], mu[:], -1.0, None, OP.mult)

                sq = ln_pool.tile([P, H], F32, tag="lnsq")
                sqs = lns_pool.tile([P, 1], F32, tag="sqs")
                nc.scalar.activation(sq[:], y[:], AF.Square,
                                     bias=nmu[:], scale=1.0,
                                     accum_out=sqs[:])
                v1 = lns_pool.tile([P, 1], F32, tag="v1")
                nc.vector.tensor_scalar(v1[:], sqs[:], 1.0 / H, LN_EPS,
                                        OP.mult, OP.add)
                v2 = lns_pool.tile([P, 1], F32, tag="v2")
                nc.vector.reciprocal(v2[:], v1[:])
                rstd = lns_pool.tile([P, 1], F32, tag="rstd")
                nc.scalar.activation(rstd[:], v2[:], AF.Sqrt)
                nmr = lns_pool.tile([P, 1], F32, tag="nmr")
                nc.vector.tensor_tensor(nmr[:], nmu[:], rstd[:], OP.mult)

                yn = ln_pool.tile([P, H], F32, tag="lnyn")
                nc.scalar.activation(yn[:], y[:], AF.Identity,
                                     bias=nmr[:], scale=rstd[:])
                if not ln_trivial:
                    nc.vector.tensor_tensor(yn[:], yn[:], lnwb[:], OP.mult)
                    nc.vector.tensor_tensor(yn[:], yn[:], lnbb[:], OP.add)
                nc.sync.dma_start(out[tb * P:(tb + 1) * P, :], yn[:])


# ======================= host side =======================================

def make_in_maps(hidden_states, attention_mask, W_qkv, b_qkv, W_dense,
                 b_dense, ln_w, ln_b):
    x = np.asarray(hidden_states, dtype=np.float32)
    mask = np.asarray(attention_mask, dtype=np.float32)
    Wq = np.asarray(W_qkv, dtype=np.float32)
    bq = np.asarray(b_qkv, dtype=np.float32)
    Wd = np.asarray(W_dense, dtype=np.float32)
    bd = np.asarray(b_dense, dtype=np.float32)
    lw = np.asarray(ln_w, dtype=np.float32)
    lb = np.asarray(ln_b, dtype=np.float32)

    in_maps = []
    for c in range(NCORES):
        b, g = c // 2, c % 2
        sl = slice(g * 1024, (g + 1) * 1024)
        wq_g = Wq[sl, :]
        wk_g = Wq[2048 + g * 1024:2048 + (g + 1) * 1024, :]
        wv_g = Wq[4096 + g * 1024:4096 + (g + 1) * 1024, :]
        bq_g = bq[sl]
        bk_g = bq[2048 + g * 1024:2048 + (g + 1) * 1024]
        bv_g = bq[4096 + g * 1024:4096 + (g + 1) * 1024]
        in_maps.append({
            "xt": np.ascontiguousarray(x[b].T),
            "xr": np.ascontiguousarray(x[b, g * 1024:(g + 1) * 1024, :]),
            "wqkt": np.ascontiguousarray(
                np.concatenate([wq_g, wk_g], axis=0).T),
            "wvt": np.ascontiguousarray(wv_g.T),
            "bqk": np.ascontiguousarray(
                np.concatenate([bq_g, bk_g]).reshape(16, P).T),
            "bv": bv_g.reshape(1, 1024).copy(),
            "wdt": np.ascontiguousarray(Wd[:, g * 1024:(g + 1) * 1024].T),
            "bdh": (bd * 0.5).reshape(1, H).copy(),
            "maskt": np.ascontiguousarray(mask[b, 0, 0, :].reshape(HB, P).T),
            "lnw": lw.reshape(1, H).copy(),
            "lnb": lb.reshape(1, H).copy(),
        })
    return in_maps


def build_flags(attention_mask, b_qkv, b_dense, ln_w, ln_b):
    return (
        bool(np.any(np.asarray(attention_mask) != 0.0)),
        bool(np.all(np.asarray(b_qkv)[:4096] == 0.0)),
        bool(np.all(np.asarray(b_qkv)[4096:] == 0.0)),
        bool(np.all(np.asarray(b_dense) == 0.0)),
        bool(np.all(np.asarray(ln_w) == 1.0) and np.all(np.asarray(ln_b) == 0.0)),
    )


def assemble_output(results):
    full = np.empty((B, S, H), dtype=np.float32)
    for c in range(NCORES):
        b, g = c // 2, c % 2
        full[b, g * 1024:(g + 1) * 1024, :] = results[c]["out"]
    return full


_CACHE = {}


def _get_program(flags):
    if flags not in _CACHE:
        _CACHE[flags] = build_program(*flags)
    return _CACHE[flags]


def _ensure_ntff_hook():
    """Provide antenv.axon_hooks (missing in this image) so trace=True can
    capture NTFF profiles through the axon PJRT plugin."""
    import types

    try:
        import antenv.axon_hooks  # noqa: F401
        return
    except ImportError:
        pass
    try:
        import antenv
    except ImportError:
        return
    mod = types.ModuleType("antenv.axon_hooks")
    holder = {"h": None}
    mod.set_axon_ntff_profile_hook = lambda h: holder.__setitem__("h", h)
    mod.get_axon_ntff_profile_hook = lambda: holder["h"]
    sys.modules["antenv.axon_hooks"] = mod
    antenv.axon_hooks = mod
    try:
        if "/root/.axon_site" not in sys.path:
            sys.path.insert(0, "/root/.axon_site")
        from trn_agent_boot.trn_boot import _ntff_profile_via_ctypes
        h = _ntff_profile_via_ctypes("/opt/axon/libaxon_pjrt.so")
        if h is not None:
            mod.set_axon_ntff_profile_hook(h)
    except Exception:
        pass


def kernel(hidden_states, attention_mask, W_qkv, b_qkv, W_dense, b_dense,
           ln_w, ln_b, trace=False):
    from concourse.bass_utils import run_bass_kernel_spmd

    flags = build_flags(attention_mask, b_qkv, b_dense, ln_w, ln_b)
    nc = _get_program(flags)
    in_maps = make_in_maps(hidden_states, attention_mask, W_qkv, b_qkv,
                           W_dense, b_dense, ln_w, ln_b)
    if trace:
        _ensure_ntff_hook()
        try:
            res = run_bass_kernel_spmd(nc, in_maps,
                                       core_ids=list(range(NCORES)),
                                       trace=True)
        except Exception as e:
            print("trace run failed (%s); retrying untraced" % e)
            res = run_bass_kernel_spmd(nc, in_maps,
                                       core_ids=list(range(NCORES)),
                                       trace=False)
    else:
        res = run_bass_kernel_spmd(nc, in_maps, core_ids=list(range(NCORES)),
                                   trace=False)
    out = assemble_output(res.results)
    kernel.last_result = res
    return out

